# revision 1
# baseline (speedup 1.0000x reference)
"""Causal self-attention (B=4, T=2048, C=1024, 16 heads) on 8 TRN2 NeuronCores.

Sharding: core c -> batch b = c//2, head-group g = c%2 (8 heads each).
Each core computes qkv projection for its heads, causal flash attention in a
"transposed scores" layout (scores[k, q], so the attention matrix never needs
an on-chip transpose for the AV matmul), and its partial output projection.
Host sums the two per-batch partials and adds b_proj.

All big matmuls in bf16 (fp32 PSUM accumulation). Softmax skips the max
subtraction (scores ~ N(0,1) here; exp cannot overflow fp32 for any plausible
input since qk/8 would need to exceed ~88).  The softmax denominator comes for
free from a ones-column appended to V; the denominator row is broadcast across
partitions on the (otherwise idle) GPSIMD engine, then fast-reciprocal +
multiply on the vector engine.
"""

import math

import numpy as np
import ml_dtypes

import concourse.bass as bass
import concourse.mybir as mybir
import concourse.tile as tile
from concourse import bacc, library_config
from concourse.bass_utils import run_bass_kernel_spmd

B, T, C = 4, 2048, 1024
N_HEAD = 16
HS = C // N_HEAD  # 64
NH_LOC = 8        # heads per core
HD = NH_LOC * HS  # 512 local head dims
N_CORES = 8

BF16 = mybir.dt.bfloat16
F32 = mybir.dt.float32
NP_BF16 = ml_dtypes.bfloat16


def build_program(t=T, repeat=1, hw_loop=0, bias_on_act=False, ostg_eng="act",
                  dve_exp_mod=0, diag_mode=None):
    """Build the per-core Bass program (SPMD: same program, per-core data).

    repeat>1 re-runs the whole body (including input DMAs) that many times,
    writing the same outputs — used only for differential hardware timing.
    """
    assert t % 512 == 0
    ct = C // 128          # 8 c-tiles (contraction tiles for qkv proj)
    mt = HD // 128         # 4 m-tiles of qt/kt (= head pairs)
    tt = t // 128          # t-tiles
    qc_n = t // 512        # q-chunks

    nc = bacc.Bacc(None, target_bir_lowering=False, debug=False)

    xt = nc.dram_tensor("xt", [128, ct, t], BF16, kind="ExternalInput")
    wqkv = nc.dram_tensor("wqkv", [128, ct, 3 * HD], BF16, kind="ExternalInput")
    wp = nc.dram_tensor("wp", [128, mt, C], BF16, kind="ExternalInput")
    bqk = nc.dram_tensor("bqk", [128, 2 * mt], F32, kind="ExternalInput")
    bv = nc.dram_tensor("bv", [1, HD], BF16, kind="ExternalInput")
    maskd = nc.dram_tensor("maskd", [128, 128], BF16, kind="ExternalInput")
    ypart = nc.dram_tensor("ypart", [t, C], BF16, kind="ExternalOutput")

    import contextlib

    with tile.TileContext(nc) as tc:
        loop_cm = tc.For_i(0, hw_loop, 1) if hw_loop else contextlib.nullcontext()
        with (
            tc.tile_pool(name="persist", bufs=1) as pp,
            tc.tile_pool(name="attp", bufs=10) as attp,
            tc.tile_pool(name="ostg", bufs=6) as ostgp,
            tc.tile_pool(name="dstg", bufs=2) as dstgp,
            tc.tile_pool(name="mm", bufs=2, space="PSUM") as mmp,
            tc.tile_pool(name="sc", bufs=2, space="PSUM") as scp,
            tc.tile_pool(name="yac", bufs=2, space="PSUM") as yacp,
        ):
            xt_s = pp.tile([128, ct, t], BF16, tag="xt")
            wqkv_s = pp.tile([128, ct, 3 * HD], BF16, tag="wqkv")
            wp_s = pp.tile([128, mt, C], BF16, tag="wp")
            bqk_s = pp.tile([128, 2 * mt], F32, tag="bqk")
            bv_s = pp.tile([1, HD], BF16, tag="bv")
            mask_s = pp.tile([128, 128], BF16, tag="mask")
            ones_s = pp.tile([1, t], BF16, tag="ones")
            qt_s = pp.tile([128, mt, t], BF16, tag="qt")
            kt_s = pp.tile([128, mt, t], BF16, tag="kt")
            v_s = pp.tile([128, tt, NH_LOC * 65], BF16, tag="v")
            yt_s = pp.tile([128, mt, t], BF16, tag="yt")
            ytacc_s = pp.tile([128, 2 * qc_n, 512], BF16, tag="ytacc")

            bvb_s = pp.tile([128, HD], F32, tag="bvb")
            bvf_s = pp.tile([1, HD], F32, tag="bvf")
            nc.gpsimd.load_library(library_config.attn)
            if diag_mode == "compute":
                for tt_ in (xt_s, wqkv_s, wp_s, bqk_s, bv_s, mask_s):
                    nc.vector.memset(tt_[:], 0.001)
            with loop_cm:
              for _rep in range(repeat):
                # ---- input DMAs ----
                th = t // 2
                if diag_mode != "compute":
                  for c in range(ct):
                    nc.sync.dma_start(xt_s[:, c, :th], xt[:, c, :th])
                    nc.sync.dma_start(wqkv_s[:, c, 2 * HD :], wqkv[:, c, 2 * HD :])
                  for c in range(ct):
                    nc.sync.dma_start(xt_s[:, c, th:], xt[:, c, th:])
                  nc.sync.dma_start(bv_s[:], bv[:])
                  for c in range(ct):
                    nc.sync.dma_start(wqkv_s[:, c, : 2 * HD], wqkv[:, c, : 2 * HD])
                  nc.sync.dma_start(bqk_s[:], bqk[:])
                  nc.sync.dma_start(mask_s[:], maskd[:])
                nc.vector.memset(ones_s[:], 1.0)
                if diag_mode == "dma":
                    continue
                nc.vector.tensor_copy(bvf_s[:], bv_s[:])
                nc.gpsimd.partition_broadcast(bvb_s[:], bvf_s[:])

                group_no = [0]

                def qkt_group(j, m, n):
                    # one psum group of the Q/K projection, c-order rotated so
                    # consecutive groups do not all wait on the last x DMA
                    dst = qt_s if j == 0 else kt_s
                    ps = mmp.tile([128, 512], F32, tag="mm")
                    rot = group_no[0] % ct
                    group_no[0] += 1
                    order = [(rot + c) % ct for c in range(ct)]
                    for idx, c in enumerate(order):
                        nc.tensor.matmul(
                            ps[:],
                            wqkv_s[:, c, j * HD + 128 * m : j * HD + 128 * (m + 1)],
                            xt_s[:, c, 512 * n : 512 * (n + 1)],
                            start=(idx == 0),
                            stop=(idx == ct - 1),
                        )
                    if bias_on_act:
                        nc.scalar.activation(
                            dst[:, m, 512 * n : 512 * (n + 1)], ps[:],
                            mybir.ActivationFunctionType.Identity,
                            bias=bqk_s[:, mt * j + m : mt * j + m + 1],
                        )
                    else:
                        nc.vector.tensor_scalar_add(
                            dst[:, m, 512 * n : 512 * (n + 1)],
                            ps[:],
                            bqk_s[:, mt * j + m : mt * j + m + 1],
                        )

                # ---- V projection: v[t, hd] (+ ones column per head) ----
                # head h: cols [65h..65h+64) = V, col 65h+64 = ones.
                for ti in range(tt):
                    ps = mmp.tile([128, 512], F32, tag="mm")
                    order = [(ti + c) % ct for c in range(ct)]
                    for idx, c in enumerate(order):
                        nc.tensor.matmul(
                            ps[:],
                            xt_s[:, c, 128 * ti : 128 * (ti + 1)],
                            wqkv_s[:, c, 2 * HD : 3 * HD],
                            start=(idx == 0),
                            stop=(idx == ct - 1),
                        )
                    vrow = v_s[:, ti, :].rearrange("p (j x) -> p j x", x=65)
                    prow = ps.rearrange("p (j x) -> p j x", x=64)
                    nc.vector.tensor_add(
                        vrow[:, :, 0:64], prow[:],
                        bvb_s.rearrange("p (j x) -> p j x", x=64))
                    nc.vector.memset(vrow[:, :, 64:65], 1.0)

                # ---- attention (per pair, with its Q/K projection just ahead) ----
                for i in range(mt):  # head pair
                    if i == 1 and diag_mode != "compute":
                        for m in range(mt):
                            nc.sync.dma_start(wp_s[:, m, :], wp[:, m, :])
                    for j in range(2):
                        for n in range(t // 512):
                            qkt_group(j, i, n)
                    for qci in reversed(range(qc_n)):
                        nk = 4 * qci + 4  # k-tiles for this q chunk
                        # both parities (heads 2i, 2i+1) processed together:
                        # scores are K=64 matmuls row-packed into the array
                        # halves via tile_position, sharing one psum tile.
                        yac_e = yacp.tile([128, 512], F32, tag="yac")
                        yac_o = yacp.tile([128, 512], F32, tag="yac")
                        yac = {0: yac_e, 1: yac_o}
                        for ki in range(nk):
                            off = max(0, (ki - 4 * qci) * 128)
                            w = 512 - off  # valid q width for this k-tile
                            sct = scp.tile([128, 1024], F32, tag="sc")
                            att = attp.tile([128, 1024], BF16, tag="att")
                            # parity slices packed contiguously: e at
                            # [off,512), o at [512,512+w) - no dead gap in exp
                            cols = {0: (off, 512), 1: (512, 512 + w)}
                            for parity in range(2):
                                c0, c1 = cols[parity]
                                nc.tensor.matmul(
                                    sct[:, c0:c1],
                                    kt_s[64 * parity : 64 * parity + 64, i,
                                         128 * ki : 128 * (ki + 1)],
                                    qt_s[64 * parity : 64 * parity + 64, i,
                                         512 * qci + off : 512 * (qci + 1)],
                                    start=True,
                                    stop=True,
                                    tile_position=(64 * parity, 0),
                                )
                            is_diag = 0 <= ki - 4 * qci <= 3
                            use_dve = (dve_exp_mod and not is_diag
                                       and ki % dve_exp_mod == 0)
                            if use_dve:
                                # Schraudolph exp on DVE: bf16 bit pattern of
                                # 2^(s*log2e/8) built by one fused affine with
                                # round-on-int16-write; ~2% RMS per element.
                                nc.vector.tensor_scalar(
                                    att[:, off : 512 + w].bitcast(mybir.dt.int16),
                                    sct[:, off : 512 + w],
                                    float(128 * math.log2(math.e) / math.sqrt(HS)),
                                    16256.0 - 5.5,
                                    mybir.AluOpType.mult, mybir.AluOpType.add,
                                )
                            else:
                                nc.scalar.activation(
                                    att[:, off : 512 + w], sct[:, off : 512 + w],
                                    mybir.ActivationFunctionType.Exp,
                                    scale=1.0 / math.sqrt(HS),
                                )
                            for parity in range(2):
                                c0, c1 = cols[parity]
                                if 0 <= ki - 4 * qci <= 3:
                                    blk = slice(c0, c0 + 128)
                                    nc.vector.tensor_mul(att[:, blk], att[:, blk],
                                                         mask_s[:])
                                h = 2 * i + parity
                                nc.tensor.matmul(
                                    yac[parity][0:65, off:512],
                                    v_s[:, ki, 65 * h : 65 * (h + 1)],
                                    att[:, c0:c1],
                                    start=(ki == 0),
                                    stop=(ki == nk - 1),
                                )
                        # evacuate both parities (bf16), then pair-wise
                        # denominator: copy rows 64 -> partition 0, broadcast
                        # across partitions on GPSIMD, fast-reciprocal, and
                        # multiply (all-bf16 for DVE 2x/4x modes).
                        for parity in range(2):
                            slot = 2 * qci + parity
                            nc.vector.tensor_copy(ytacc_s[0:65, slot, :],
                                                  yac[parity][0:65, :])
                        pr = slice(2 * qci, 2 * qci + 2)
                        dn = dstgp.tile([1, 2, 512], F32, tag="dn")
                        nc.vector.tensor_copy(dn[:], ytacc_s[64:65, pr, :])
                        dbc = dstgp.tile([64, 2, 512], F32, tag="dbc")
                        nc.gpsimd.partition_broadcast(dbc[:], dn[:])
                        recb = dstgp.tile([64, 2, 512], F32, tag="recb")
                        nc.vector.reciprocal_approx_fast(recb[:], dbc[:])
                        nc.vector.tensor_mul(
                            yt_s[0:64, i, 512 * qci : 512 * (qci + 1)],
                            ytacc_s[0:64, 2 * qci, :], recb[:, 0, :],
                        )
                        ytmp = dstgp.tile([64, 512], BF16, tag="ytmp")
                        nc.vector.tensor_mul(
                            ytmp[:], ytacc_s[0:64, 2 * qci + 1, :], recb[:, 1, :])
                        nc.sync.dma_start(
                            yt_s[64:128, i, 512 * qci : 512 * (qci + 1)],
                            ytmp[:])

                # ---- output projection: ypart[t, C] = y[t, hd] @ wp ----
                # t-chunks 4.. first: their yt columns (q-chunks >= 1) are
                # normalized before q-chunk 0 (processed last under desc order)
                for ti in list(range(4, tt)) + list(range(4)):
                    ostg = ostgp.tile([128, C], BF16, tag="ostg")
                    for cc in range(C // 512):
                        ps = mmp.tile([128, 512], F32, tag="mm")
                        for i in range(mt):
                            nc.tensor.matmul(
                                ps[:],
                                yt_s[:, i, 128 * ti : 128 * (ti + 1)],
                                wp_s[:, i, 512 * cc : 512 * (cc + 1)],
                                start=(i == 0),
                                stop=(i == mt - 1),
                            )
                        on_dve = (ostg_eng == "dve" or (ostg_eng == "mix" and cc % 2 == 0))
                        if on_dve:
                            nc.vector.tensor_copy(ostg[:, 512 * cc : 512 * (cc + 1)], ps[:])
                        else:
                            nc.scalar.copy(ostg[:, 512 * cc : 512 * (cc + 1)], ps[:])
                    nc.sync.dma_start(ypart[128 * ti : 128 * (ti + 1), :], ostg[:])

    nc.compile()
    return nc


_PROGRAM_CACHE = {}


def _get_program(t=T):
    if t not in _PROGRAM_CACHE:
        _PROGRAM_CACHE[t] = build_program(t)
    return _PROGRAM_CACHE[t]


def make_in_maps(x, W_attn, b_attn, W_proj, b_proj, t=T):
    ct = C // 128
    mt = HD // 128
    mask = np.greater_equal(np.arange(128)[None, :], np.arange(128)[:, None])
    mask_bf = mask.astype(NP_BF16)
    in_maps = []
    for core in range(N_CORES):
        b = core // 2
        g = core % 2
        h0 = g * NH_LOC
        cs = h0 * HS          # 512*g : column start within each of q/k/v
        # [C, t] -> [128, ct, t] (c-tile-major partition layout)
        xt_np = np.ascontiguousarray(
            x[b].T.reshape(ct, 128, t).transpose(1, 0, 2)).astype(NP_BF16)
        wq = W_attn[:, cs : cs + HD]
        wk = W_attn[:, C + cs : C + cs + HD]
        wv = W_attn[:, 2 * C + cs : 2 * C + cs + HD]
        wqkv_np = np.concatenate([wq, wk, wv], axis=1).astype(NP_BF16)
        wqkv_np = np.ascontiguousarray(
            wqkv_np.reshape(ct, 128, 3 * HD).transpose(1, 0, 2))
        wp_np = W_proj[cs : cs + HD, :].astype(NP_BF16)
        wp_np = np.ascontiguousarray(
            wp_np.reshape(mt, 128, C).transpose(1, 0, 2))
        bq = b_attn[cs : cs + HD]
        bk = b_attn[C + cs : C + cs + HD]
        bv_ = b_attn[2 * C + cs : 2 * C + cs + HD]
        bqk_np = np.concatenate(
            [bq.reshape(mt, 128).T, bk.reshape(mt, 128).T], axis=1
        ).astype(np.float32)
        in_maps.append({
            "xt": xt_np,
            "wqkv": wqkv_np,
            "wp": wp_np,
            "bqk": np.ascontiguousarray(bqk_np),
            "bv": bv_.reshape(1, HD).astype(NP_BF16),
            "maskd": mask_bf,
        })
    return in_maps


def combine_outputs(results, b_proj, t=T):
    out = np.empty((B, t, C), dtype=np.float32)
    for b in range(B):
        out[b] = results[2 * b]["ypart"].astype(np.float32)
        out[b] += results[2 * b + 1]["ypart"].astype(np.float32)
        out[b] += b_proj[None, :]
    return out


def kernel(x, W_attn, b_attn, W_proj, b_proj):
    x = np.asarray(x, dtype=np.float32)
    W_attn = np.asarray(W_attn, dtype=np.float32)
    b_attn = np.asarray(b_attn, dtype=np.float32)
    W_proj = np.asarray(W_proj, dtype=np.float32)
    b_proj = np.asarray(b_proj, dtype=np.float32)
    nc = _get_program(T)
    in_maps = make_in_maps(x, W_attn, b_attn, W_proj, b_proj, T)
    res = run_bass_kernel_spmd(nc, in_maps, core_ids=list(range(N_CORES)))
    return combine_outputs(res.results, b_proj)



# revision 33
# speedup vs baseline: 1.1336x; 1.1336x over previous
"""Causal self-attention (B=4, T=2048, C=1024, 16 heads) on 8 TRN2 NeuronCores.

Sharding: core c -> batch b = c//2, head-group g = c%2 (8 heads each).
Each core computes the qkv projection for its heads, causal flash attention,
and its partial output projection; the host sums the two per-batch partials
and adds b_proj.

Attention layout: scores are computed transposed (scores[k, q], k on psum
partitions) so the attention matrix never needs an on-chip transpose for the
AV matmul.  The AV matmul uses the exp'd score block as the STATIONARY
operand and V (with an appended ones-column) as the moving operand:
out[q, 0:65] = att[k,q].T @ [V | 1].  This (a) streams only 65 columns per
128x128 score block instead of 128 (half the AV tensor-engine time), and
(b) lands the softmax denominator on column 64 *per partition*, so
normalization is a per-partition reciprocal+multiply on DVE -- no partition
broadcast.  Normalized [q, head-pair] tiles are transposed back to [hd, q]
on the tensor engine (128-cycle transposes through a shared psum slot).

All big matmuls in bf16 (fp32 PSUM accumulation).  Softmax skips the max
subtraction (scores ~ N(0,1); exp cannot overflow fp32 here).

Scheduling: q-chunks ascend so pair 0 starts after only the first quarter of
x is loaded; the next pair's Q/K projection groups and the V projection are
interleaved into each pair's attention stream as PE filler; the output
projection is interleaved into pair 3's attention the same way.  ~20 warmup
matmuls run during the initial DMA wait to lift the PE HAM clock gate before
real work arrives.
"""

import math

import numpy as np
import ml_dtypes

import concourse.bass as bass
import concourse.mybir as mybir
import concourse.tile as tile
from concourse import bacc
from concourse.bass_utils import run_bass_kernel_spmd

B, T, C = 4, 2048, 1024
N_HEAD = 16
HS = C // N_HEAD  # 64
NH_LOC = 8        # heads per core
HD = NH_LOC * HS  # 512 local head dims
N_CORES = 8

BF16 = mybir.dt.bfloat16
F32 = mybir.dt.float32
NP_BF16 = ml_dtypes.bfloat16


def build_program(t=T, repeat=1, hw_loop=0, ostg_eng="mix", norm_eng="dve",
                  dve_exp_mod=3, mask_eng="pool", diag_mode=None, dbg=False):
    """Build the per-core Bass program (SPMD: same program, per-core data)."""
    assert t % 512 == 0
    ct = C // 128          # 8 c-tiles (contraction tiles for qkv proj)
    mt = HD // 128         # 4 m-tiles (head pairs)
    tt = t // 128          # t-tiles
    qc_n = t // 512        # q-chunks

    nc = bacc.Bacc(None, target_bir_lowering=False, debug=False)

    xt = nc.dram_tensor("xt", [128, ct, t], BF16, kind="ExternalInput")
    wqkv = nc.dram_tensor("wqkv", [128, ct, 3 * HD], BF16, kind="ExternalInput")
    wp = nc.dram_tensor("wp", [128, mt, C], BF16, kind="ExternalInput")
    bqk = nc.dram_tensor("bqk", [128, 2 * mt], F32, kind="ExternalInput")
    bv = nc.dram_tensor("bv", [1, HD], BF16, kind="ExternalInput")
    maskd = nc.dram_tensor("maskd", [128, 128], BF16, kind="ExternalInput")
    identd = nc.dram_tensor("identd", [128, 128], BF16, kind="ExternalInput")
    ypart = nc.dram_tensor("ypart", [t, C], BF16, kind="ExternalOutput")
    if dbg:
        qt_d = nc.dram_tensor("qt_d", [128, HD // 128, t], BF16,
                              kind="ExternalOutput")
        kt_d = nc.dram_tensor("kt_d", [128, HD // 128, t], BF16,
                              kind="ExternalOutput")
        v_d = nc.dram_tensor("v_d", [128, t // 128, NH_LOC * 65], BF16,
                             kind="ExternalOutput")
        yt_d = nc.dram_tensor("yt_d", [128, HD // 128, t], BF16,
                              kind="ExternalOutput")

    import contextlib

    with tile.TileContext(nc) as tc:
        loop_cm = tc.For_i(0, hw_loop, 1) if hw_loop else contextlib.nullcontext()
        with (
            tc.tile_pool(name="persist", bufs=1) as pp,
            tc.tile_pool(name="attp", bufs=10) as attp,
            tc.tile_pool(name="nstg", bufs=4) as nstgp,
            tc.tile_pool(name="ostg", bufs=6) as ostgp,
            tc.tile_pool(name="mm", bufs=2, space="PSUM") as mmp,
            tc.tile_pool(name="sc", bufs=2, space="PSUM") as scp,
            tc.tile_pool(name="yq", bufs=1, space="PSUM") as yqp,
        ):
            xt_s = pp.tile([128, ct, t], BF16, tag="xt")
            wqkv_s = pp.tile([128, ct, 3 * HD], BF16, tag="wqkv")
            wp_s = pp.tile([128, mt, C], BF16, tag="wp")
            bqk_s = pp.tile([128, 2 * mt], F32, tag="bqk")
            bv_s = pp.tile([1, HD], BF16, tag="bv")
            mask_s = pp.tile([128, 128], BF16, tag="mask")
            ident_s = pp.tile([128, 128], BF16, tag="ident")
            qt_s = pp.tile([128, mt, t], BF16, tag="qt")
            kt_s = pp.tile([128, mt, t], BF16, tag="kt")
            v_s = pp.tile([128, tt, NH_LOC * 65], BF16, tag="v")
            yt_s = pp.tile([128, mt, t], BF16, tag="yt")
            wtiny_s = pp.tile([128, 512], BF16, tag="wtiny")

            bvb_s = pp.tile([128, HD], F32, tag="bvb")
            bvf_s = pp.tile([1, HD], F32, tag="bvf")

            if diag_mode == "compute":
                for tt_ in (xt_s, wqkv_s, wp_s, bqk_s, bv_s, mask_s, ident_s):
                    nc.vector.memset(tt_[:], 0.001)
            with loop_cm:
              for _rep in range(repeat):
                # ---- input DMAs (t-quarter granularity for early start) ----
                if diag_mode != "compute":
                    # one coarse DMA per x-quarter / weight slab: each DMA
                    # pays a fixed ~625ns HWDGE queue slot, so fine-grained
                    # DMAs serialize the head of the kernel.
                    nc.sync.dma_start(xt_s[:, :, :512], xt[:, :, :512])
                    nc.sync.dma_start(wqkv_s[:, :, 0:256], wqkv[:, :, 0:256])
                    nc.sync.dma_start(bqk_s[:], bqk[:])
                    nc.sync.dma_start(bv_s[:], bv[:])
                    nc.sync.dma_start(wqkv_s[:, :, 4 * 256:],
                                      wqkv[:, :, 4 * 256:])
                    nc.sync.dma_start(mask_s[:], maskd[:])
                    nc.sync.dma_start(ident_s[:], identd[:])
                    for n in range(1, max(qc_n, mt)):
                        if n < qc_n:
                            nc.sync.dma_start(
                                xt_s[:, :, 512 * n: 512 * (n + 1)],
                                xt[:, :, 512 * n: 512 * (n + 1)])
                        if n < mt:
                            nc.sync.dma_start(
                                wqkv_s[:, :, 256 * n: 256 * (n + 1)],
                                wqkv[:, :, 256 * n: 256 * (n + 1)])

                nc.vector.memset(wtiny_s[:], 0.001)
                nc.vector.tensor_copy(bvf_s[:], bv_s[:])
                nc.gpsimd.partition_broadcast(bvb_s[:], bvf_s[:])

                # ---- PE warmup: lift the HAM clock gate during DMA wait ----
                for wi in range(12):
                    wps = mmp.tile([128, 512], F32, tag="mm")
                    nc.tensor.matmul(wps[0:16, :], wtiny_s[:, 0:16], wtiny_s[:],
                                     start=True, stop=True)
                if diag_mode == "dma":
                    continue

                def qkt_group(j, m, n, bias_eng="dve"):
                    # one psum group of the Q or K projection
                    dst = qt_s if j == 0 else kt_s
                    ps = mmp.tile([128, 512], F32, tag="mm")
                    for idx in range(ct):
                        nc.tensor.matmul(
                            ps[:],
                            wqkv_s[:, idx, 256 * m + 128 * j:
                                   256 * m + 128 * (j + 1)],
                            xt_s[:, idx, 512 * n: 512 * (n + 1)],
                            start=(idx == 0),
                            stop=(idx == ct - 1),
                        )
                    if bias_eng == "act":
                        nc.scalar.activation(
                            dst[:, m, 512 * n: 512 * (n + 1)], ps[:],
                            mybir.ActivationFunctionType.Identity,
                            bias=bqk_s[:, mt * j + m: mt * j + m + 1],
                        )
                    else:
                        nc.vector.tensor_scalar_add(
                            dst[:, m, 512 * n: 512 * (n + 1)],
                            ps[:],
                            bqk_s[:, mt * j + m: mt * j + m + 1],
                        )

                def v_group(ti):
                    # V projection for t-tile ti: v[t, hd] + ones col per head
                    ps = mmp.tile([128, 512], F32, tag="mm")
                    for idx in range(ct):
                        nc.tensor.matmul(
                            ps[:],
                            xt_s[:, idx, 128 * ti: 128 * (ti + 1)],
                            wqkv_s[:, idx, 4 * 256: 4 * 256 + HD],
                            start=(idx == 0),
                            stop=(idx == ct - 1),
                        )
                    vrow = v_s[:, ti, :].rearrange("p (j x) -> p j x", x=65)
                    prow = ps.rearrange("p (j x) -> p j x", x=64)
                    nc.vector.tensor_add(
                        vrow[:, :, 0:64], prow[:],
                        bvb_s.rearrange("p (j x) -> p j x", x=64))
                    nc.vector.memset(vrow[:, :, 64:65], 1.0)

                def op_cc(ti, cc):
                    # half of the output projection for t-tile ti
                    ostg = ostgp.tile([128, 512], BF16, tag="ostg")
                    ps = mmp.tile([128, 512], F32, tag="mm")
                    for i2 in range(mt):
                        nc.tensor.matmul(
                            ps[:],
                            yt_s[:, i2, 128 * ti: 128 * (ti + 1)],
                            wp_s[:, i2, 512 * cc: 512 * (cc + 1)],
                            start=(i2 == 0),
                            stop=(i2 == mt - 1),
                        )
                    if ostg_eng == "dve" or (ostg_eng == "mix" and cc == 0):
                        nc.vector.tensor_copy(ostg[:], ps[:])
                    else:
                        nc.scalar.copy(ostg[:], ps[:])
                    nc.sync.dma_start(
                        ypart[128 * ti: 128 * (ti + 1),
                              512 * cc: 512 * (cc + 1)], ostg[:])

                def op_group(ti):
                    op_cc(ti, 0)
                    op_cc(ti, 1)

                def tr_copy(i, qci, j, ystg):
                    # transpose [q, hd-pair] -> [hd, q] for q-tile j via the
                    # XBAR DMA transpose: no tensor-engine or DVE time, no
                    # psum slot -- just a DMA queue slot.
                    tj = 4 * qci + j
                    nc.sync.dma_start_transpose(
                        yt_s[:, i, 128 * tj: 128 * (tj + 1)], ystg[:, j, :])

                def norm_chunk(yq, ystg, sl, n=4):
                    # per-partition softmax denominators -> reciprocal -> mul
                    # (slots sl..sl+n of yq into matching ystg columns, or
                    # all 8 slots when sl < 0)
                    if sl < 0:
                        n = 8
                    s0 = max(sl, 0)
                    rec = nstgp.tile([128, 8, 1], F32, tag="rec", bufs=8)
                    recsl = rec[:, s0:s0 + n, :]
                    nc.vector.reciprocal_approx_fast(
                        recsl, yq[:, s0:s0 + n, 64:65])
                    ysv = ystg.rearrange("p a (b x) -> p (a b) x", x=64)
                    recb = bass.AP(recsl.tensor, recsl.offset,
                                   recsl.ap[:-1] + [[0, 64]])
                    nc.vector.tensor_mul(ysv[:, s0:s0 + n, :],
                                         yq[:, s0:s0 + n, 0:64], recb)

                def emit_scores_exp(i, qci, ki, nk):
                    d = ki - 4 * qci  # >=0 on the diagonal
                    off = max(0, d * 128)
                    w = 512 - off
                    sct = scp.tile([128, 1024], F32, tag="sc")
                    att = attp.tile([128, 1024], BF16, tag="att")
                    cols = {0: (off, 512), 1: (512, 512 + w)}
                    for parity in range(2):
                        c0, c1 = cols[parity]
                        nc.tensor.matmul(
                            sct[:, c0:c1],
                            kt_s[64 * parity: 64 * parity + 64, i,
                                 128 * ki: 128 * (ki + 1)],
                            qt_s[64 * parity: 64 * parity + 64, i,
                                 512 * qci + off: 512 * (qci + 1)],
                            start=True,
                            stop=True,
                            tile_position=(64 * parity, 0),
                        )
                    use_dve = (dve_exp_mod and d < 0 and ki < nk - 3
                               and ki % dve_exp_mod == 1)
                    if use_dve:
                        # Schraudolph exp on DVE: bf16 bit pattern of
                        # 2^(s*log2e/8) via one fused affine with
                        # round-on-int16-write; ~2% RMS per element.
                        nc.vector.tensor_scalar(
                            att[:, off: 512 + w].bitcast(mybir.dt.int16),
                            sct[:, off: 512 + w],
                            float(128 * math.log2(math.e) / math.sqrt(HS)),
                            16256.0 - 5.5,
                            mybir.AluOpType.mult,
                            mybir.AluOpType.add,
                        )
                    else:
                        nc.scalar.activation(
                            att[:, off: 512 + w], sct[:, off: 512 + w],
                            mybir.ActivationFunctionType.Exp,
                            scale=1.0 / math.sqrt(HS),
                        )
                    return att

                def mask_av(i, qci, ki, att, yq):
                    d = ki - 4 * qci
                    off = max(0, d * 128)
                    if d >= 0:
                        # mask the diagonal block of each parity (SBUF-only
                        # op: legal on the Pool engine, which is idle)
                        meng = nc.gpsimd if mask_eng == "pool" else nc.vector
                        meng.tensor_mul(
                            att[:, off: off + 128],
                            att[:, off: off + 128], mask_s[:])
                        meng.tensor_mul(
                            att[:, 512: 640],
                            att[:, 512: 640], mask_s[:])
                    # AV: att block stationary, V moving.  PSUM accumulation
                    # groups are per 2KB bank (slots 0-3 = bank 0, 4-7 =
                    # bank 1): start only on the bank's first write of the
                    # chunk (marks the whole bank pending-zero, so each
                    # slot's first write overwrites), stop on its last.
                    for j in range(max(0, d), 4):
                        for parity in range(2):
                            a0 = 128 * j if parity == 0 \
                                else 512 + 128 * j - off
                            h = 2 * i + parity
                            nc.tensor.matmul(
                                yq[:, 2 * j + parity, 0:65],
                                att[:, a0: a0 + 128],
                                v_s[:, ki, 65 * h: 65 * (h + 1)],
                                start=(ki == 0 and parity == 0
                                       and j in (0, 2)),
                                stop=(parity == 1
                                      and ((j == 1 and ki == 4 * qci + 1)
                                           or (j == 3 and ki == 4 * qci + 3))),
                            )

                # ---- attention: flat strip stream, chunk-outer pair-inner --
                # AV lags the scores/exp emission by one strip so the exp
                # latency is always covered by the next strip's scores.
                # Chunk-outer ordering lets the output projection of chunk c
                # (ready once pair 3 finishes c) weave into chunk c+1 across
                # all pairs.  Two work queues feed the gaps: urgent (V / QK
                # projections with near deadlines, up to 2 per strip) and
                # lazy (transposes + output projection, 1 per strip).
                strips = [(qci, i, ki)
                          for qci in range(qc_n)
                          for i in range(mt)
                          for ki in range(4 * qci + 4)]
                sidx = {s: n for n, s in enumerate(strips)}

                def vdl(ti_):
                    # deadline for V t-tile ti_: the strip emitting its
                    # first AV consumer (one strip after (qci', 0, ti_))
                    q2 = ti_ // 4
                    n = sidx[(q2, 0, ti_)] + 1
                    return n

                urgent = []  # (cost_ns, deadline_strip, fn)
                lazy = []
                prev = None  # (i, qci, ki, att, yq, ystg)
                state = {}
                GRP_NS = 1707   # 8x512-cycle projection group
                OP_NS = 853     # 4x512-cycle output projection half
                TR_NS = 80      # transpose

                def finalize(p):
                    # AV of a completed strip + per-chunk finalization
                    i, qci, ki, att, yq, ystg = p
                    mask_av(i, qci, ki, att, yq)
                    d = ki - 4 * qci
                    last = i == mt - 1
                    if last and qci == qc_n - 1 and d in (1, 3):
                        # very tail: normalize/transpose/project per psum
                        # bank as its accumulation group stops (bank 0 =
                        # q-tiles 0-1 at d==1, bank 1 = q-tiles 2-3 at
                        # d==3).  All still-queued transposes must be
                        # emitted before the projection reads yt.
                        while lazy:
                            lazy.pop(0)[1]()
                        norm_chunk(yq, ystg, 4 * (d // 2), n=4)
                        for j2 in (d - 1, d):
                            tr_copy(i, qci, j2, ystg)
                            op_group(4 * qci + j2)
                    elif ki == 4 * qci + 3:
                        # chunk done: normalization now (DVE, overlaps the
                        # next strips); transposes + OP into the lazy queue.
                        norm_chunk(yq, ystg, -1)
                        for j in range(4):
                            lazy.append(
                                (TR_NS,
                                 lambda i=i, qci=qci, j=j, ystg=ystg:
                                 tr_copy(i, qci, j, ystg)))
                        if last:
                            for j in range(4):
                                for cc in range(2):
                                    lazy.append(
                                        (OP_NS,
                                         lambda qci=qci, j=j, cc=cc:
                                         op_cc(4 * qci + j, cc)))

                balance = 0.0
                for snum, (qci, i, ki) in enumerate(strips):
                    if ki == 0:
                        # chunk entry: queue urgent fillers, each with the
                        # strip index by which it must have been emitted
                        if qci == 0 and i == 1 and diag_mode != "compute":
                            nc.sync.dma_start(wp_s[:], wp[:])
                        if i == 0 and qci == 0:
                            # pair-0 projections must precede the first
                            # scores in program order (in-order engines)
                            qkt_group(0, 0, 0, bias_eng="act")
                            qkt_group(1, 0, 0, bias_eng="act")
                            for ti_ in range(0, 4):
                                urgent.append(
                                    (GRP_NS, vdl(ti_),
                                     lambda ti_=ti_: v_group(ti_)))
                        if i < mt - 1:
                            dl = sidx[(qci, i + 1, 0)]
                            urgent.append(
                                (GRP_NS, dl, lambda i=i, qci=qci:
                                 qkt_group(0, i + 1, qci)))
                            urgent.append(
                                (GRP_NS, dl, lambda i=i, qci=qci:
                                 qkt_group(1, i + 1, qci)))
                        if i == 1 and qci < qc_n - 1:
                            for ti_ in range(4 * qci + 4, 4 * qci + 8):
                                urgent.append(
                                    (GRP_NS, vdl(ti_),
                                     lambda ti_=ti_: v_group(ti_)))
                        if i == 2 and qci < qc_n - 1:
                            n = qci + 1
                            dl = sidx[(n, 0, 0)]
                            urgent.append(
                                (GRP_NS, dl, lambda n=n: qkt_group(0, 0, n)))
                            urgent.append(
                                (GRP_NS, dl, lambda n=n: qkt_group(1, 0, n)))
                        state['yq'] = yqp.tile([128, 8, 128], F32,
                                               tag="yq", name="yq")
                        state['ystg'] = nstgp.tile([128, 4, 128], BF16,
                                                   tag="ystg", name="ystg")
                    urgent.sort(key=lambda it: it[1])
                    # anything due by this strip is emitted before its scores
                    while urgent and urgent[0][1] <= snum:
                        cost, _, fn = urgent.pop(0)
                        fn()
                        balance -= cost
                    att = emit_scores_exp(i, qci, ki, 4 * qci + 4)
                    # fill the exp latency of this strip with queued PE work
                    # (~700ns/strip beyond the next scores + lagged AV)
                    balance += 700.0
                    while (urgent or lazy) and balance > 0:
                        if urgent:
                            cost, _, fn = urgent.pop(0)
                        else:
                            cost, fn = lazy.pop(0)
                        fn()
                        balance -= cost
                    balance = min(max(balance, -3500.0), 1400.0)
                    if prev is not None:
                        finalize(prev)
                    prev = (i, qci, ki, att, state['yq'], state['ystg'])
                finalize(prev)
                for _, _, fn in urgent:
                    fn()
                for _, fn in lazy:
                    fn()
                if dbg:
                    nc.sync.dma_start(qt_d[:], qt_s[:])
                    nc.sync.dma_start(kt_d[:], kt_s[:])
                    nc.sync.dma_start(v_d[:], v_s[:])
                    nc.sync.dma_start(yt_d[:], yt_s[:])

    nc.compile()
    return nc


_PROGRAM_CACHE = {}


def _get_program(t=T):
    if t not in _PROGRAM_CACHE:
        _PROGRAM_CACHE[t] = build_program(t)
    return _PROGRAM_CACHE[t]


def make_in_maps(x, W_attn, b_attn, W_proj, b_proj, t=T):
    ct = C // 128
    mt = HD // 128
    mask = np.greater_equal(np.arange(128)[None, :], np.arange(128)[:, None])
    mask_bf = mask.astype(NP_BF16)
    ident_bf = np.eye(128, dtype=NP_BF16)
    in_maps = []
    for core in range(N_CORES):
        b = core // 2
        g = core % 2
        h0 = g * NH_LOC
        cs = h0 * HS          # 512*g : column start within each of q/k/v
        # [C, t] -> [128, ct, t] (c-tile-major partition layout)
        xt_np = np.ascontiguousarray(
            x[b].T.reshape(ct, 128, t).transpose(1, 0, 2)).astype(NP_BF16)
        wq = W_attn[:, cs: cs + HD]
        wk = W_attn[:, C + cs: C + cs + HD]
        wv = W_attn[:, 2 * C + cs: 2 * C + cs + HD]
        # column slabs: [q_m | k_m] per m-tile, then v
        slabs = []
        for m in range(mt):
            slabs.append(wq[:, 128 * m: 128 * (m + 1)])
            slabs.append(wk[:, 128 * m: 128 * (m + 1)])
        slabs.append(wv)
        wqkv_np = np.concatenate(slabs, axis=1).astype(NP_BF16)
        wqkv_np = np.ascontiguousarray(
            wqkv_np.reshape(ct, 128, 3 * HD).transpose(1, 0, 2))
        wp_np = W_proj[cs: cs + HD, :].astype(NP_BF16)
        wp_np = np.ascontiguousarray(
            wp_np.reshape(mt, 128, C).transpose(1, 0, 2))
        bq = b_attn[cs: cs + HD]
        bk = b_attn[C + cs: C + cs + HD]
        bv_ = b_attn[2 * C + cs: 2 * C + cs + HD]
        bqk_np = np.concatenate(
            [bq.reshape(mt, 128).T, bk.reshape(mt, 128).T], axis=1
        ).astype(np.float32)
        in_maps.append({
            "xt": xt_np,
            "wqkv": wqkv_np,
            "wp": wp_np,
            "bqk": np.ascontiguousarray(bqk_np),
            "bv": bv_.reshape(1, HD).astype(NP_BF16),
            "maskd": mask_bf,
            "identd": ident_bf,
        })
    return in_maps


def combine_outputs(results, b_proj, t=T):
    out = np.empty((B, t, C), dtype=np.float32)
    for b in range(B):
        out[b] = results[2 * b]["ypart"].astype(np.float32)
        out[b] += results[2 * b + 1]["ypart"].astype(np.float32)
        out[b] += b_proj[None, :]
    return out


def kernel(x, W_attn, b_attn, W_proj, b_proj):
    x = np.asarray(x, dtype=np.float32)
    W_attn = np.asarray(W_attn, dtype=np.float32)
    b_attn = np.asarray(b_attn, dtype=np.float32)
    W_proj = np.asarray(W_proj, dtype=np.float32)
    b_proj = np.asarray(b_proj, dtype=np.float32)
    nc = _get_program(T)
    in_maps = make_in_maps(x, W_attn, b_attn, W_proj, b_proj, T)
    res = run_bass_kernel_spmd(nc, in_maps, core_ids=list(range(N_CORES)))
    return combine_outputs(res.results, b_proj)
